# revision 5
# baseline (speedup 1.0000x reference)
"""Trainium2 Bass kernel for nn_MEMOIRWrapper (MEMOIR-style edit wrapper), v2.

Straight-line program per batch b (data-parallel over 8 cores, weights
replicated); the GEMM never waits on the mask phase:

    out      = x @ W.T  (+ bias added on host)            # PE only
    agg      = sum_{s<=boundary} x[s,:]                   # DVE, from a
               (reduced over the free axis of a d-major     dedicated fp8
                fp8 copy of x that streams in early)        x stream
    v        = |agg - n*bg_mean|; top-K threshold via 16-way count-exact
               search (DVE + 1 tiny PE all-reduce matmul per iteration)
    overlap  -> relevant flag + best saved mask fm        # tiny aux output

The `relevant` branch (never taken for random masks: best overlap ratio
~0.125 << 0.3 threshold) is resolved on the HOST: if the aux flag fires,
the host adds (x * fm) @ nW.T in f32.  The device program is branch-free,
so PE streams 2048 dense bf16 matmuls (437 us roofline) back to back.

Layouts are host-pretiled so every big DMA is contiguous per partition:
x bf16 d-major tiles (one 512KB DMA per s-tile), weights slab-major (one
2MB DMA per 512-wide o-slab), fp8 x for the prefix sum.  The mask-phase
PE ops are emitted interleaved into the GEMM loop (search iter i after
s-tile 4+i) so they sit early in the in-order PE queue, right where
their DVE dependencies are already resolved — zero PE stalls.
PSUM evictions run on ACT/Pool so DVE does only mask work.
"""

from contextlib import ExitStack

import numpy as np
import ml_dtypes

import concourse.bass as bass
from concourse import bacc
import concourse.mybir as mybir
from concourse.bass import ds, ts  # noqa: F401
from concourse.bass_utils import run_bass_kernel_spmd
from concourse.tile import TileContext

F32 = mybir.dt.float32
BF16 = mybir.dt.bfloat16
FP8 = mybir.dt.float8e4
I32 = mybir.dt.int32
ALU = mybir.AluOpType
AX = mybir.AxisListType

B, S, D, O = 8, 4096, 2048, 2048
M = 128
P = 128
NS = S // P          # 32 s-tiles
ND = D // P          # 16 d-tiles
NQ = O // 512        # 4 o-chains per s-tile
SLAB = ND * 512
NWAY = 16            # thresholds tested per search iteration
SEARCH_ITERS = 6     # 1024/17^6 ~ 4e-5, far below the top-K gap
N_CORES = 8

# scal layout ([1, 8] f32):
#   0: boundary (clipped)  1: n = boundary+1  2: K-0.5  3: unused
#   4: relevant_count_thr - 0.5               5..7: unused


def _build_program():
    nc = bacc.Bacc("TRN2", target_bir_lowering=False, debug=False)

    # xT_d[p, (s*ND + j)*P + c] = x[s*P + c, j*P + p]   (bf16, GEMM)
    xT_d = nc.dram_tensor("xT", [P, NS * ND * P], BF16, kind="ExternalInput")
    # x8T_d: same layout, fp8e4 (prefix-sum stream)
    x8T_d = nc.dram_tensor("x8T", [P, NS * ND * P], FP8, kind="ExternalInput")
    # wt2_d[p, (oq*ND + j)*512 + c] = W[oq*512 + c, j*P + p]
    wt2_d = nc.dram_tensor("wt2", [P, NQ * SLAB], BF16, kind="ExternalInput")
    bg_d = nc.dram_tensor("bg_r", [1, D], F32, kind="ExternalInput")
    savedT_d = nc.dram_tensor("savedT", [P, ND * M], BF16, kind="ExternalInput")
    savedPT_d = nc.dram_tensor("savedPT", [P, ND * M], BF16,
                               kind="ExternalInput")
    fracs_d = nc.dram_tensor("fracs", [1, SEARCH_ITERS * NWAY], F32,
                             kind="ExternalInput")
    mrow_d = nc.dram_tensor("mrow", [1, M], F32, kind="ExternalInput")
    scal_d = nc.dram_tensor("scal", [1, 8], F32, kind="ExternalInput")
    out_d = nc.dram_tensor("out", [S, O], BF16, kind="ExternalOutput")
    aux_d = nc.dram_tensor("aux", [P, ND + 2], F32, kind="ExternalOutput")

    with TileContext(nc) as tc, ExitStack() as top:
        # ---------------- constants ----------------
        const = top.enter_context(tc.tile_pool(name="const", bufs=1))

        onesPP_f = const.tile([P, P], F32, tag="onesPP")
        nc.vector.memset(onesPP_f[:], 1.0)
        ones_row_f = const.tile([1, P], F32, tag="onesrowf")
        nc.vector.memset(ones_row_f[:], 1.0)

        scal8_bc = const.tile([P, 8], F32, tag="scal8bc")
        b_bc = scal8_bc[:, 0:1]
        n_bc = scal8_bc[:, 1:2]
        km_bc = scal8_bc[:, 2:3]

        pos_i = const.tile([P, S], I32, tag="posi")
        pm_bc = const.tile([P, S], BF16, tag="pmbc")

        bg16 = const.tile([P, ND], F32, tag="bg16")
        savedT = const.tile([P, ND * M], BF16, tag="savedT")
        savedPT = const.tile([P, ND * M], BF16, tag="savedPT")
        iota2d_sm = const.tile([P, M], F32, tag="iota2dsm")
        iotam4096 = const.tile([1, M], F32, tag="iotam4096")
        fracS = const.tile([P, SEARCH_ITERS * NWAY], F32, tag="fracS")

        wpool = top.enter_context(tc.tile_pool(name="weffT", bufs=1))
        w_all = wpool.tile([P, NQ * SLAB], BF16, tag="wall")

        sb = top.enter_context(tc.tile_pool(name="sbsmall", bufs=1))

        # -------- DMA order: slab0 + first xT tiles ride ahead, then the
        # other slabs, then the fp8 prefix stream, then consts.
        xTp = top.enter_context(tc.tile_pool(name="xT", bufs=8))

        def load_xt(s):
            t = xTp.tile([P, ND * P], BF16, tag="xT", name=f"xT{s}")
            nc.sync.dma_start(
                t[:], xT_d[:, s * ND * P:(s + 1) * ND * P])
            return t

        # first x tile + weight slab 0 arrive in quarters so the first
        # GEMM chain starts ~2us in
        xT0 = xTp.tile([P, ND * P], BF16, tag="xT", name="xT0")
        Hx = ND * P // 4
        Hw = SLAB // 4
        for q in range(4):
            nc.sync.dma_start(xT0[:, q * Hx:(q + 1) * Hx],
                              xT_d[:, q * Hx:(q + 1) * Hx])
            nc.sync.dma_start(w_all[:, q * Hw:(q + 1) * Hw],
                              wt2_d[:, q * Hw:(q + 1) * Hw])
        xT_tiles = {0: xT0, 1: load_xt(1), 2: load_xt(2)}
        nc.sync.dma_start(scal8_bc[:], scal_d[0:1, :].to_broadcast((P, 8)))
        # pm_bc[p, s] = (s <= boundary) on every partition (the prefix
        # mask applies along the FREE axis of the d-major x tiles).
        nc.gpsimd.iota(pos_i[:], pattern=[[1, S]], base=0,
                       channel_multiplier=0)
        nc.vector.tensor_scalar(pm_bc[:], pos_i[:], b_bc, None,
                                op0=ALU.is_le)
        # the slab0-only warm chains eat xT tiles 0-5 first; slabs 1-3
        # are not needed until the catch-up chains (~22us+)
        xT_tiles[3] = load_xt(3)
        xT_tiles[4] = load_xt(4)
        xT_tiles[5] = load_xt(5)
        nc.sync.dma_start(w_all[:, SLAB:2 * SLAB], wt2_d[:, SLAB:2 * SLAB])
        nc.sync.dma_start(w_all[:, 2 * SLAB:3 * SLAB],
                          wt2_d[:, 2 * SLAB:3 * SLAB])
        nc.sync.dma_start(w_all[:, 3 * SLAB:4 * SLAB],
                          wt2_d[:, 3 * SLAB:4 * SLAB])
        xT_tiles[6] = load_xt(6)
        xT_tiles[7] = load_xt(7)

        # -------- pools ------------------------------------------------
        ps_out_pool = top.enter_context(
            tc.tile_pool(name="ps_out", bufs=5, space="PSUM"))
        ps_sm = top.enter_context(
            tc.tile_pool(name="ps_sm", bufs=1, space="PSUM"))
        outp = top.enter_context(tc.tile_pool(name="outsb", bufs=4))
        x8p = top.enter_context(tc.tile_pool(name="x8", bufs=4))
        prodp = top.enter_context(tc.tile_pool(name="prod", bufs=2))
        partp = top.enter_context(tc.tile_pool(name="part", bufs=2))
        mks = top.enter_context(tc.tile_pool(name="mks", bufs=1))

        # -------- prefix sum on DVE from the fp8 stream ----------------
        # (x8 loads are paced one-per-chain inside the GEMM loop so the
        #  fp8 stream doesn't compete with weights/xT for DMA bandwidth)
        agg16 = sb.tile([P, ND], F32, tag="agg16")
        nc.vector.memset(agg16[:], 0.0)

        def emit_prefix_step(s):
            x8 = x8p.tile([P, ND * P], FP8, tag="x8", name=f"x8_{s}")
            nc.sync.dma_start(
                x8[:], x8T_d[:, s * ND * P:(s + 1) * ND * P])
            prod = prodp.tile([P, ND, P], FP8, tag="prod")
            nc.vector.tensor_tensor(
                prod[:], x8[:].rearrange("p (j c) -> p j c", j=ND),
                pm_bc[:, s * P:(s + 1) * P].unsqueeze(1)
                .to_broadcast((P, ND, P)),
                op=ALU.mult)
            part = partp.tile([P, ND], F32, tag="part")
            nc.vector.reduce_sum(part[:], prod[:], axis=AX.X)
            nc.vector.tensor_tensor(agg16[:], agg16[:], part[:], op=ALU.add)

        def emit_mask_consts():
            nc.sync.dma_start(
                bg16[:], bg_d[0:1, :].rearrange("a (f p) -> (a p) f", p=P))
            nc.sync.dma_start(
                fracS[:],
                fracs_d[0:1, :].to_broadcast((P, SEARCH_ITERS * NWAY)))
            nc.sync.dma_start(savedPT[:], savedPT_d[:, :])
            nc.sync.dma_start(savedT[:], savedT_d[:, :])
            nc.sync.dma_start(iota2d_sm[:],
                              mrow_d[0:1, :].to_broadcast((P, M)))
            nc.sync.dma_start(iotam4096[:], mrow_d[0:1, :])

        v16 = mks.tile([P, ND], F32, tag="v16")
        lo_box = [None]

        def emit_v16():
            # v16 = |agg - n*bg| (d-major folded [P, ND])
            nbg16 = mks.tile([P, ND], F32, tag="nbg16")
            nc.vector.tensor_tensor(
                nbg16[:], bg16[:], n_bc.to_broadcast((P, ND)), op=ALU.mult)
            nc.vector.tensor_tensor(v16[:], agg16[:], nbg16[:],
                                    op=ALU.subtract)
            vneg = mks.tile([P, ND], F32, tag="vneg")
            nc.vector.tensor_scalar(vneg[:], v16[:], -1.0, None,
                                    op0=ALU.mult)
            nc.vector.tensor_tensor(v16[:], v16[:], vneg[:], op=ALU.max)
            lo0 = sb.tile([P, 1], F32, tag="lo", bufs=2)
            nc.vector.memset(lo0[:], 0.0)
            lo_box[0] = lo0

        def emit_search_iter(it):
            # count-exact threshold search, constant shrink schedule:
            # bracket [lo, lo + HI0/17^it] always contains v_(K); final
            # width 1024/17^6 ~ 4e-5 << the K-th order-statistic gap.
            lo = lo_box[0]
            fr = fracS[:, it * NWAY:(it + 1) * NWAY]
            mid8 = mks.tile([P, NWAY], F32, tag="mid8", bufs=2)
            nc.vector.tensor_scalar(
                mid8[:], fr, lo[:, 0:1], None, op0=ALU.add)
            ge8 = mks.tile([P, NWAY, ND], F32, tag="ge8", bufs=2)
            nc.vector.tensor_tensor(
                ge8[:],
                v16[:].unsqueeze(1).to_broadcast((P, NWAY, ND)),
                mid8[:].unsqueeze(2).to_broadcast((P, NWAY, ND)),
                op=ALU.is_ge)
            cnt_p8 = mks.tile([P, NWAY], F32, tag="cntp8", bufs=2)
            nc.vector.reduce_sum(cnt_p8[:], ge8[:], axis=AX.X)
            cnt_ps = ps_sm.tile([P, NWAY], F32, tag="cntps")
            nc.tensor.matmul(cnt_ps[:], onesPP_f[:], cnt_p8[:],
                             start=True, stop=True)
            geK8 = mks.tile([P, NWAY], F32, tag="geK8", bufs=2)
            nc.vector.tensor_tensor(
                geK8[:], cnt_ps[:], km_bc.to_broadcast((P, NWAY)),
                op=ALU.is_ge)
            t2 = mks.tile([P, NWAY], F32, tag="t2", bufs=2)
            nc.vector.tensor_tensor(t2[:], fr, geK8[:], op=ALU.mult)
            lomax = mks.tile([P, 1], F32, tag="lomax", bufs=2)
            nc.vector.reduce_max(lomax[:], t2[:], axis=AX.X)
            lo2 = sb.tile([P, 1], F32, tag="lo", bufs=2)
            nc.vector.tensor_tensor(lo2[:], lo[:], lomax[:], op=ALU.add)
            lo_box[0] = lo2

        ov_ps_box = [None]

        def emit_overlap():
            lo = lo_box[0]
            ind16 = mks.tile([P, ND], BF16, tag="ind16")
            nc.vector.tensor_tensor(
                ind16[:], v16[:], lo[:].to_broadcast((P, ND)), op=ALU.is_ge)
            ov_ps = ps_sm.tile([1, M], F32, tag="row_ps")
            for t in range(ND):
                nc.tensor.matmul(
                    ov_ps[:],
                    ind16[:, t:t + 1],
                    savedPT[:, t * M:(t + 1) * M],
                    start=(t == 0),
                    stop=(t == ND - 1),
                )
            ov_ps_box[0] = ov_ps

        def emit_fm_aux():
            ov_ps = ov_ps_box[0]
            maxo = mks.tile([1, 1], F32, tag="maxo")
            nc.vector.tensor_reduce(maxo[:], ov_ps[:], axis=AX.X, op=ALU.max)
            rel01 = mks.tile([1, 1], F32, tag="rel01")
            nc.vector.tensor_tensor(
                rel01[:], maxo[:], scal8_bc[0:1, 4:5], op=ALU.is_ge)
            # best = first argmax: min over eqm*(m-4096)
            eqm = mks.tile([1, M], F32, tag="eqm")
            nc.vector.tensor_tensor(
                eqm[:], ov_ps[:], maxo[:].to_broadcast((1, M)), op=ALU.is_ge)
            cand = mks.tile([1, M], F32, tag="cand")
            nc.vector.tensor_tensor(cand[:], eqm[:], iotam4096[:],
                                    op=ALU.mult)
            best = mks.tile([1, 1], F32, tag="best")
            nc.vector.tensor_reduce(best[:], cand[:], axis=AX.X, op=ALU.min)
            bc_ps = ps_sm.tile([P, 1], F32, tag="bc_ps")
            nc.tensor.matmul(bc_ps[:], ones_row_f[:], best[:],
                             start=True, stop=True)
            best_bc = mks.tile([P, 1], F32, tag="bestbc")
            nc.vector.tensor_copy(best_bc[:], bc_ps[:])
            # fm16[p, t] = savedT[p, t*128+best]  (one-hot dot, exact)
            ohrep = mks.tile([P, M], BF16, tag="ohrep")
            nc.vector.tensor_tensor(
                ohrep[:], iota2d_sm[:], best_bc[:].to_broadcast((P, M)),
                op=ALU.is_equal)
            t5 = mks.tile([P, ND, M], BF16, tag="t5")
            nc.vector.tensor_tensor(
                t5[:],
                savedT[:].rearrange("p (t m) -> p t m", t=ND),
                ohrep[:].unsqueeze(1).to_broadcast((P, ND, M)),
                op=ALU.mult)
            fm16 = mks.tile([P, ND], BF16, tag="fm16")
            with nc.allow_low_precision(
                    reason="0/1 one-hot dot, exact in bf16"):
                nc.vector.reduce_sum(fm16[:], t5[:], axis=AX.X)
            aux_sb = mks.tile([P, ND + 2], F32, tag="auxsb")
            nc.vector.memset(aux_sb[:], 0.0)
            nc.vector.tensor_copy(aux_sb[:, 0:ND], fm16[:])
            nc.vector.tensor_copy(aux_sb[0:1, ND:ND + 1], rel01[:])
            nc.vector.tensor_copy(aux_sb[0:1, ND + 1:ND + 2], best[:])
            nc.sync.dma_start(aux_d[:, :], aux_sb[:])

        # -------- GEMM: chain schedule keeps PE dense from ~3.5us ------
        # First WARM s-tiles run only their oq=0 chain (slab 0) while
        # slabs 1-3 stream in; then the skipped chains catch up.
        WARM = 6
        chain_order = [(s, 0) for s in range(WARM)]
        chain_order += [(s, oq) for oq in range(1, NQ) for s in range(WARM)]
        chain_order += [(s, oq) for s in range(WARM, NS) for oq in range(NQ)]
        # mask-phase ops are emitted at these chain indices: the prefix
        # finishes by ci=31 (PE ~110us), so deps are comfortably ready
        mask_at = {32: lambda: (emit_mask_consts(), emit_v16())}
        for i in range(SEARCH_ITERS):
            mask_at[34 + 2 * i] = lambda i=i: emit_search_iter(i)
        mask_at[34 + 2 * SEARCH_ITERS + 2] = emit_overlap
        mask_at[34 + 2 * SEARCH_ITERS + 10] = emit_fm_aux

        for ci, (s, oq) in enumerate(chain_order):
            if s not in xT_tiles:
                xT_tiles[s] = load_xt(s)
            xT = xT_tiles[s]
            if ci < NS:
                emit_prefix_step(ci)
            # last s-tile: half-width chains so the final
            # evict->store tail is as short as possible
            nhalf = 2 if s == NS - 1 else 1
            cw = 512 // nhalf
            for h in range(nhalf):
                po = ps_out_pool.tile([P, 512], F32, tag="outps")
                for j in range(ND):
                    ocol = (oq * ND + j) * 512 + h * cw
                    nc.tensor.matmul(
                        po[:, 0:cw],
                        xT[:, j * P:(j + 1) * P],
                        w_all[:, ocol:ocol + cw],
                        start=(j == 0),
                        stop=(j == ND - 1),
                    )
                osb = outp.tile([P, 512], BF16, tag="osb")
                # GPSIMD cannot read PSUM, and DVE evictions would queue
                # behind the prefix chain (priority inversion starving
                # PSUM) -> all evictions on ACT; DVE only at the tail
                if s == NS - 1:
                    nc.vector.tensor_copy(osb[:, 0:cw], po[:, 0:cw])
                else:
                    nc.scalar.copy(osb[:, 0:cw], po[:, 0:cw])
                nc.sync.dma_start(
                    out_d[s * P:(s + 1) * P,
                          oq * 512 + h * cw:oq * 512 + (h + 1) * cw],
                    osb[:, 0:cw])
            if ci in mask_at:
                mask_at[ci]()

    nc.compile()
    return nc


_PROGRAM = None


def _get_program():
    global _PROGRAM
    if _PROGRAM is None:
        _PROGRAM = _build_program()
    return _PROGRAM


def _rel_count_threshold(k: int) -> float:
    kf = np.float32(k)
    thr = np.float32(0.3)
    for c in range(k + 2):
        if np.float32(c) / kf >= thr:
            return float(c)
    return float(k + 1)


def _make_in_maps(x, boundaries, weight, bias, new_weight, permutation,
                  saved_masks, bg_mean, top_k):
    bf16 = ml_dtypes.bfloat16
    fp8 = ml_dtypes.float8_e4m3
    x = np.asarray(x, dtype=np.float32)
    boundaries = np.asarray(boundaries)
    w = np.asarray(weight, dtype=np.float32)
    # wt2[p, (oq*ND + j)*512 + c] = W[oq*512 + c, j*P + p]
    wt2 = np.ascontiguousarray(
        w.astype(bf16).reshape(NQ, 512, ND, P)
        .transpose(3, 0, 2, 1).reshape(P, NQ * SLAB))
    bg = np.ascontiguousarray(
        np.asarray(bg_mean, dtype=np.float32).reshape(1, D))
    perm = np.asarray(permutation).astype(np.int64)
    saved = np.asarray(saved_masks).astype(np.float32)        # [M, D]
    # savedT[p, t*128+m]  = saved[m, t*128+p]
    savedT = np.ascontiguousarray(
        saved.T.reshape(ND, P, M).transpose(1, 0, 2).reshape(P, ND * M)
        .astype(bf16))
    # savedPT[p, t*128+m] = saved[m, perm[t*128+p]]
    savedPT = np.ascontiguousarray(
        saved[:, perm].T.reshape(ND, P, M).transpose(1, 0, 2)
        .reshape(P, ND * M).astype(bf16))
    HI0 = 1024.0
    fracs = np.ascontiguousarray(np.array(
        [[(k + 1) / (NWAY + 1) * HI0 / (NWAY + 1) ** it
          for it in range(SEARCH_ITERS) for k in range(NWAY)]],
        dtype=np.float32))
    mrow = np.ascontiguousarray(
        (np.arange(M, dtype=np.float32) - 4096.0).reshape(1, M))
    k = int(top_k)
    relc = _rel_count_threshold(k)

    in_maps = []
    for i in range(N_CORES):
        bnd = float(np.clip(int(boundaries[i]), 0, S - 1))
        scal = np.array(
            [[bnd, bnd + 1.0, k - 0.5, 0.0, relc - 0.5, 0.0, 0.0, 0.0]],
            dtype=np.float32)
        # xT[p, s, j, c] = x[s*P + c, j*P + p]
        xb = x[i].astype(bf16)
        xt = np.ascontiguousarray(
            xb.reshape(NS, P, ND, P).transpose(3, 0, 2, 1)
            .reshape(P, NS * ND * P))
        x8t = np.ascontiguousarray(
            x[i].astype(fp8).reshape(NS, P, ND, P).transpose(3, 0, 2, 1)
            .reshape(P, NS * ND * P))
        in_maps.append({
            "xT": xt,
            "x8T": x8t,
            "wt2": wt2,
            "bg_r": bg,
            "savedT": savedT,
            "savedPT": savedPT,
            "fracs": fracs,
            "mrow": mrow,
            "scal": scal,
        })
    return in_maps


def run(inputs: dict, trace: bool = False):
    nc = _get_program()
    in_maps = _make_in_maps(**inputs)
    res = run_bass_kernel_spmd(
        nc, in_maps, core_ids=list(range(N_CORES)), trace=trace)
    bias = np.asarray(inputs["bias"], dtype=np.float32)
    nw = np.asarray(inputs["new_weight"], dtype=np.float32)
    x = np.asarray(inputs["x"], dtype=np.float32)
    outs = []
    for i in range(N_CORES):
        o = np.asarray(res.results[i]["out"]).astype(np.float32) + bias
        aux = np.asarray(res.results[i]["aux"]).astype(np.float32)
        if aux[0, ND] != 0.0:
            # relevant: add the masked new-weight path (host f32, exact)
            fm = aux[:, 0:ND].T.reshape(D)  # fm[t*128+p] = aux[p, t]
            o = o + (x[i] * fm[None, :]) @ nw.T
        outs.append(o)
    return np.stack(outs, axis=0), res


def kernel(**inputs) -> np.ndarray:
    out, _ = run(inputs, trace=False)
    return out


# revision 7
# speedup vs baseline: 1.0938x; 1.0938x over previous
"""Trainium2 Bass kernel for nn_MEMOIRWrapper (MEMOIR-style edit wrapper), v2.

Straight-line program per batch b (data-parallel over 8 cores, weights
replicated); the GEMM never waits on the mask phase:

    out      = x @ W.T  (+ bias added on host)            # PE only
    agg      = sum_{s<=boundary} x[s,:]                   # DVE, from a
               (reduced over the free axis of a d-major     dedicated fp8
                fp8 copy of x that streams in early)        x stream
    v        = |agg - n*bg_mean|; top-K threshold via 16-way count-exact
               search (DVE + 1 tiny PE all-reduce matmul per iteration)
    overlap  -> relevant flag + best saved mask fm        # tiny aux output

The `relevant` branch (never taken for random masks: best overlap ratio
~0.125 << 0.3 threshold) is resolved on the HOST: if the aux flag fires,
the host adds (x * fm) @ nW.T in f32.  The device program is branch-free,
so PE streams 2048 dense bf16 matmuls (437 us roofline) back to back.

Layouts are host-pretiled so every big DMA is contiguous per partition:
x bf16 d-major tiles (one 512KB DMA per s-tile), weights slab-major (one
2MB DMA per 512-wide o-slab), fp8 x for the prefix sum.  The mask-phase
PE ops are emitted interleaved into the GEMM loop (search iter i after
s-tile 4+i) so they sit early in the in-order PE queue, right where
their DVE dependencies are already resolved — zero PE stalls.
PSUM evictions run on ACT/Pool so DVE does only mask work.
"""

from contextlib import ExitStack

import numpy as np
import ml_dtypes

import concourse.bass as bass
from concourse import bacc
import concourse.mybir as mybir
from concourse.bass import ds, ts  # noqa: F401
from concourse.bass_utils import run_bass_kernel_spmd
from concourse.tile import TileContext

F32 = mybir.dt.float32
BF16 = mybir.dt.bfloat16
FP8 = mybir.dt.float8e4
I32 = mybir.dt.int32
ALU = mybir.AluOpType
AX = mybir.AxisListType

B, S, D, O = 8, 4096, 2048, 2048
M = 128
P = 128
NS = S // P          # 32 s-tiles
ND = D // P          # 16 d-tiles
NDB = ND - 2         # d-tiles 0..13 run bf16; 14,15 run fp8 DoubleRow
NQ = O // 512        # 4 o-chains per s-tile
SLAB = NDB * 512
NWAY = 16            # thresholds tested per search iteration
SEARCH_ITERS = 6     # 1024/17^6 ~ 4e-5, far below the top-K gap
N_CORES = 8
DR_SCALE = 0.141     # x*a (x), W/a (w): balances both into e4m3's
                     # normal range; product is exact so no descale

# scal layout ([1, 8] f32):
#   0: boundary (clipped)  1: n = boundary+1  2: K-0.5  3: unused
#   4: relevant_count_thr - 0.5               5..7: unused


def _build_program():
    nc = bacc.Bacc("TRN2", target_bir_lowering=False, debug=False)

    # xT_d[p, (s*NDB + j)*P + c] = x[s*P + c, j*P + p]  (bf16, d-tiles 0..13)
    xT_d = nc.dram_tensor("xT", [P, NS * NDB * P], BF16, kind="ExternalInput")
    # xdr_d[p, (s*2 + i)*P + c] = fp8(a * x[s*P + c, (14+i)*P + p])
    xdr_d = nc.dram_tensor("xdr", [P, NS * 2 * P], FP8, kind="ExternalInput")
    # x8T_d: full-D fp8 d-major tiles (prefix-sum stream)
    x8T_d = nc.dram_tensor("x8T", [P, NS * ND * P], FP8, kind="ExternalInput")
    # wt2_d[p, (oq*NDB + j)*512 + c] = W[oq*512 + c, j*P + p]
    wt2_d = nc.dram_tensor("wt2", [P, NQ * SLAB], BF16, kind="ExternalInput")
    # wdr_d[p, (oq*2 + i)*512 + n] = fp8(W[oq*512 + n, (14+i)*P + p] / a)
    wdr_d = nc.dram_tensor("wdr", [P, NQ * 2 * 512], FP8, kind="ExternalInput")
    bg_d = nc.dram_tensor("bg_r", [1, D], F32, kind="ExternalInput")
    savedT_d = nc.dram_tensor("savedT", [P, ND * M], BF16, kind="ExternalInput")
    savedPT_d = nc.dram_tensor("savedPT", [P, ND * M], BF16,
                               kind="ExternalInput")
    fracs_d = nc.dram_tensor("fracs", [1, SEARCH_ITERS * NWAY], F32,
                             kind="ExternalInput")
    mrow_d = nc.dram_tensor("mrow", [1, M], F32, kind="ExternalInput")
    scal_d = nc.dram_tensor("scal", [1, 8], F32, kind="ExternalInput")
    out_d = nc.dram_tensor("out", [S, O], BF16, kind="ExternalOutput")
    aux_d = nc.dram_tensor("aux", [P, ND + 2], F32, kind="ExternalOutput")

    with TileContext(nc) as tc, ExitStack() as top:
        # ---------------- constants ----------------
        const = top.enter_context(tc.tile_pool(name="const", bufs=1))

        onesPP_f = const.tile([P, P], F32, tag="onesPP")
        nc.vector.memset(onesPP_f[:], 1.0)
        ones_row_f = const.tile([1, P], F32, tag="onesrowf")
        nc.vector.memset(ones_row_f[:], 1.0)

        scal8_bc = const.tile([P, 8], F32, tag="scal8bc")
        b_bc = scal8_bc[:, 0:1]
        n_bc = scal8_bc[:, 1:2]
        km_bc = scal8_bc[:, 2:3]

        pos_i = const.tile([P, S], I32, tag="posi")
        pm_bc = const.tile([P, S], BF16, tag="pmbc")

        bg16 = const.tile([P, ND], F32, tag="bg16")
        savedT = const.tile([P, ND * M], BF16, tag="savedT")
        savedPT = const.tile([P, ND * M], BF16, tag="savedPT")
        iota2d_sm = const.tile([P, M], F32, tag="iota2dsm")
        iotam4096 = const.tile([1, M], F32, tag="iotam4096")
        fracS = const.tile([P, SEARCH_ITERS * NWAY], F32, tag="fracS")

        wpool = top.enter_context(tc.tile_pool(name="weffT", bufs=1))
        w_all = wpool.tile([P, NQ * SLAB], BF16, tag="wall")
        # native 3D tiles for the DoubleRow operands (pair dim explicit)
        wdr_t = [wpool.tile([P, 2, 512], FP8, tag=f"wdr{oq}",
                            name=f"wdr{oq}")
                 for oq in range(NQ)]

        sb = top.enter_context(tc.tile_pool(name="sbsmall", bufs=1))

        # -------- DMA order: slab0 + first xT tiles ride ahead, then the
        # other slabs, then the fp8 prefix stream, then consts.
        xTp = top.enter_context(tc.tile_pool(name="xT", bufs=8))
        xdrp = top.enter_context(tc.tile_pool(name="xdr", bufs=8))

        def load_xdr(s):
            t = xdrp.tile([P, 2, P], FP8, tag="xdr", name=f"xdr{s}")
            nc.sync.dma_start(
                t[:],
                xdr_d[:, s * 2 * P:(s + 1) * 2 * P]
                .rearrange("p (i c) -> p i c", i=2))
            return t

        def load_xt(s):
            t = xTp.tile([P, NDB * P], BF16, tag="xT", name=f"xT{s}")
            nc.sync.dma_start(
                t[:], xT_d[:, s * NDB * P:(s + 1) * NDB * P])
            return t, load_xdr(s)

        # first x tile + weight slab 0 arrive in quarters so the first
        # GEMM chain starts ~2us in
        xT0 = xTp.tile([P, NDB * P], BF16, tag="xT", name="xT0")
        Hx = NDB * P // 4
        Hw = SLAB // 4
        for q in range(4):
            nc.sync.dma_start(xT0[:, q * Hx:(q + 1) * Hx],
                              xT_d[:, q * Hx:(q + 1) * Hx])
            nc.sync.dma_start(w_all[:, q * Hw:(q + 1) * Hw],
                              wt2_d[:, q * Hw:(q + 1) * Hw])
        for oq in range(NQ):
            nc.sync.dma_start(
                wdr_t[oq][:],
                wdr_d[:, oq * 1024:(oq + 1) * 1024]
                .rearrange("p (i n) -> p i n", i=2))
        xT_tiles = {0: (xT0, load_xdr(0)), 1: load_xt(1), 2: load_xt(2)}
        nc.sync.dma_start(scal8_bc[:], scal_d[0:1, :].to_broadcast((P, 8)))
        # pm_bc[p, s] = (s <= boundary) on every partition (the prefix
        # mask applies along the FREE axis of the d-major x tiles).
        nc.gpsimd.iota(pos_i[:], pattern=[[1, S]], base=0,
                       channel_multiplier=0)
        nc.vector.tensor_scalar(pm_bc[:], pos_i[:], b_bc, None,
                                op0=ALU.is_le)
        # the slab0-only warm chains eat xT tiles 0-5 first; slabs 1-3
        # are not needed until the catch-up chains (~22us+)
        xT_tiles[3] = load_xt(3)
        xT_tiles[4] = load_xt(4)
        xT_tiles[5] = load_xt(5)
        nc.sync.dma_start(w_all[:, SLAB:2 * SLAB], wt2_d[:, SLAB:2 * SLAB])
        nc.sync.dma_start(w_all[:, 2 * SLAB:3 * SLAB],
                          wt2_d[:, 2 * SLAB:3 * SLAB])
        nc.sync.dma_start(w_all[:, 3 * SLAB:4 * SLAB],
                          wt2_d[:, 3 * SLAB:4 * SLAB])
        xT_tiles[6] = load_xt(6)
        xT_tiles[7] = load_xt(7)

        # -------- pools ------------------------------------------------
        ps_out_pool = top.enter_context(
            tc.tile_pool(name="ps_out", bufs=5, space="PSUM"))
        ps_sm = top.enter_context(
            tc.tile_pool(name="ps_sm", bufs=1, space="PSUM"))
        outp = top.enter_context(tc.tile_pool(name="outsb", bufs=4))
        x8p = top.enter_context(tc.tile_pool(name="x8", bufs=4))
        prodp = top.enter_context(tc.tile_pool(name="prod", bufs=2))
        partp = top.enter_context(tc.tile_pool(name="part", bufs=2))
        mks = top.enter_context(tc.tile_pool(name="mks", bufs=1))

        # -------- prefix sum on DVE from the fp8 stream ----------------
        # (x8 loads are paced one-per-chain inside the GEMM loop so the
        #  fp8 stream doesn't compete with weights/xT for DMA bandwidth)
        agg16 = sb.tile([P, ND], F32, tag="agg16")
        nc.vector.memset(agg16[:], 0.0)

        def emit_prefix_step(s):
            x8 = x8p.tile([P, ND * P], FP8, tag="x8", name=f"x8_{s}")
            nc.sync.dma_start(
                x8[:], x8T_d[:, s * ND * P:(s + 1) * ND * P])
            prod = prodp.tile([P, ND, P], FP8, tag="prod")
            nc.vector.tensor_tensor(
                prod[:], x8[:].rearrange("p (j c) -> p j c", j=ND),
                pm_bc[:, s * P:(s + 1) * P].unsqueeze(1)
                .to_broadcast((P, ND, P)),
                op=ALU.mult)
            part = partp.tile([P, ND], F32, tag="part")
            nc.vector.reduce_sum(part[:], prod[:], axis=AX.X)
            nc.vector.tensor_tensor(agg16[:], agg16[:], part[:], op=ALU.add)

        def emit_mask_consts():
            nc.sync.dma_start(
                bg16[:], bg_d[0:1, :].rearrange("a (f p) -> (a p) f", p=P))
            nc.sync.dma_start(
                fracS[:],
                fracs_d[0:1, :].to_broadcast((P, SEARCH_ITERS * NWAY)))
            nc.sync.dma_start(savedPT[:], savedPT_d[:, :])
            nc.sync.dma_start(savedT[:], savedT_d[:, :])
            nc.sync.dma_start(iota2d_sm[:],
                              mrow_d[0:1, :].to_broadcast((P, M)))
            nc.sync.dma_start(iotam4096[:], mrow_d[0:1, :])

        v16 = mks.tile([P, ND], F32, tag="v16")
        lo_box = [None]

        def emit_v16():
            # v16 = |agg - n*bg| (d-major folded [P, ND])
            nbg16 = mks.tile([P, ND], F32, tag="nbg16")
            nc.vector.tensor_tensor(
                nbg16[:], bg16[:], n_bc.to_broadcast((P, ND)), op=ALU.mult)
            nc.vector.tensor_tensor(v16[:], agg16[:], nbg16[:],
                                    op=ALU.subtract)
            vneg = mks.tile([P, ND], F32, tag="vneg")
            nc.vector.tensor_scalar(vneg[:], v16[:], -1.0, None,
                                    op0=ALU.mult)
            nc.vector.tensor_tensor(v16[:], v16[:], vneg[:], op=ALU.max)
            lo0 = sb.tile([P, 1], F32, tag="lo", bufs=2)
            nc.vector.memset(lo0[:], 0.0)
            lo_box[0] = lo0

        def emit_search_iter(it):
            # count-exact threshold search, constant shrink schedule:
            # bracket [lo, lo + HI0/17^it] always contains v_(K); final
            # width 1024/17^6 ~ 4e-5 << the K-th order-statistic gap.
            lo = lo_box[0]
            fr = fracS[:, it * NWAY:(it + 1) * NWAY]
            mid8 = mks.tile([P, NWAY], F32, tag="mid8", bufs=2)
            nc.vector.tensor_scalar(
                mid8[:], fr, lo[:, 0:1], None, op0=ALU.add)
            ge8 = mks.tile([P, NWAY, ND], F32, tag="ge8", bufs=2)
            nc.vector.tensor_tensor(
                ge8[:],
                v16[:].unsqueeze(1).to_broadcast((P, NWAY, ND)),
                mid8[:].unsqueeze(2).to_broadcast((P, NWAY, ND)),
                op=ALU.is_ge)
            cnt_p8 = mks.tile([P, NWAY], F32, tag="cntp8", bufs=2)
            nc.vector.reduce_sum(cnt_p8[:], ge8[:], axis=AX.X)
            cnt_ps = ps_sm.tile([P, NWAY], F32, tag="cntps")
            nc.tensor.matmul(cnt_ps[:], onesPP_f[:], cnt_p8[:],
                             start=True, stop=True)
            geK8 = mks.tile([P, NWAY], F32, tag="geK8", bufs=2)
            nc.vector.tensor_tensor(
                geK8[:], cnt_ps[:], km_bc.to_broadcast((P, NWAY)),
                op=ALU.is_ge)
            t2 = mks.tile([P, NWAY], F32, tag="t2", bufs=2)
            nc.vector.tensor_tensor(t2[:], fr, geK8[:], op=ALU.mult)
            lomax = mks.tile([P, 1], F32, tag="lomax", bufs=2)
            nc.vector.reduce_max(lomax[:], t2[:], axis=AX.X)
            lo2 = sb.tile([P, 1], F32, tag="lo", bufs=2)
            nc.vector.tensor_tensor(lo2[:], lo[:], lomax[:], op=ALU.add)
            lo_box[0] = lo2

        ov_ps_box = [None]

        def emit_overlap():
            lo = lo_box[0]
            ind16 = mks.tile([P, ND], BF16, tag="ind16")
            nc.vector.tensor_tensor(
                ind16[:], v16[:], lo[:].to_broadcast((P, ND)), op=ALU.is_ge)
            ov_ps = ps_sm.tile([1, M], F32, tag="row_ps")
            for t in range(ND):
                nc.tensor.matmul(
                    ov_ps[:],
                    ind16[:, t:t + 1],
                    savedPT[:, t * M:(t + 1) * M],
                    start=(t == 0),
                    stop=(t == ND - 1),
                )
            ov_ps_box[0] = ov_ps

        def emit_fm_aux():
            ov_ps = ov_ps_box[0]
            maxo = mks.tile([1, 1], F32, tag="maxo")
            nc.vector.tensor_reduce(maxo[:], ov_ps[:], axis=AX.X, op=ALU.max)
            rel01 = mks.tile([1, 1], F32, tag="rel01")
            nc.vector.tensor_tensor(
                rel01[:], maxo[:], scal8_bc[0:1, 4:5], op=ALU.is_ge)
            # best = first argmax: min over eqm*(m-4096)
            eqm = mks.tile([1, M], F32, tag="eqm")
            nc.vector.tensor_tensor(
                eqm[:], ov_ps[:], maxo[:].to_broadcast((1, M)), op=ALU.is_ge)
            cand = mks.tile([1, M], F32, tag="cand")
            nc.vector.tensor_tensor(cand[:], eqm[:], iotam4096[:],
                                    op=ALU.mult)
            best = mks.tile([1, 1], F32, tag="best")
            nc.vector.tensor_reduce(best[:], cand[:], axis=AX.X, op=ALU.min)
            bc_ps = ps_sm.tile([P, 1], F32, tag="bc_ps")
            nc.tensor.matmul(bc_ps[:], ones_row_f[:], best[:],
                             start=True, stop=True)
            best_bc = mks.tile([P, 1], F32, tag="bestbc")
            nc.vector.tensor_copy(best_bc[:], bc_ps[:])
            # fm16[p, t] = savedT[p, t*128+best]  (one-hot dot, exact)
            ohrep = mks.tile([P, M], BF16, tag="ohrep")
            nc.vector.tensor_tensor(
                ohrep[:], iota2d_sm[:], best_bc[:].to_broadcast((P, M)),
                op=ALU.is_equal)
            t5 = mks.tile([P, ND, M], BF16, tag="t5")
            nc.vector.tensor_tensor(
                t5[:],
                savedT[:].rearrange("p (t m) -> p t m", t=ND),
                ohrep[:].unsqueeze(1).to_broadcast((P, ND, M)),
                op=ALU.mult)
            fm16 = mks.tile([P, ND], BF16, tag="fm16")
            with nc.allow_low_precision(
                    reason="0/1 one-hot dot, exact in bf16"):
                nc.vector.reduce_sum(fm16[:], t5[:], axis=AX.X)
            aux_sb = mks.tile([P, ND + 2], F32, tag="auxsb")
            nc.vector.memset(aux_sb[:], 0.0)
            nc.vector.tensor_copy(aux_sb[:, 0:ND], fm16[:])
            nc.vector.tensor_copy(aux_sb[0:1, ND:ND + 1], rel01[:])
            nc.vector.tensor_copy(aux_sb[0:1, ND + 1:ND + 2], best[:])
            nc.sync.dma_start(aux_d[:, :], aux_sb[:])

        # -------- GEMM: chain schedule keeps PE dense from ~3.5us ------
        # First WARM s-tiles run only their oq=0 chain (slab 0) while
        # slabs 1-3 stream in; then the skipped chains catch up.
        WARM = 6
        chain_order = [(s, 0) for s in range(WARM)]
        chain_order += [(s, oq) for oq in range(1, NQ) for s in range(WARM)]
        chain_order += [(s, oq) for s in range(WARM, NS) for oq in range(NQ)]
        # mask-phase ops are emitted at these chain indices: the prefix
        # finishes by ci=31 (PE ~110us), so deps are comfortably ready
        mask_at = {32: lambda: (emit_mask_consts(), emit_v16())}
        for i in range(SEARCH_ITERS):
            mask_at[34 + 2 * i] = lambda i=i: emit_search_iter(i)
        mask_at[34 + 2 * SEARCH_ITERS + 2] = emit_overlap
        mask_at[34 + 2 * SEARCH_ITERS + 10] = emit_fm_aux

        for ci, (s, oq) in enumerate(chain_order):
            if s not in xT_tiles:
                xT_tiles[s] = load_xt(s)
            xT, xdr = xT_tiles[s]
            if ci < NS:
                emit_prefix_step(ci)
            # last s-tile: half-width chains so the final
            # evict->store tail is as short as possible
            nhalf = 2 if s == NS - 1 else 1
            cw = 512 // nhalf
            for h in range(nhalf):
                po = ps_out_pool.tile([P, 512], F32, tag="outps")
                for j in range(NDB):
                    ocol = (oq * NDB + j) * 512 + h * cw
                    nc.tensor.matmul(
                        po[:, 0:cw],
                        xT[:, j * P:(j + 1) * P],
                        w_all[:, ocol:ocol + cw],
                        start=(j == 0),
                        stop=False,
                    )
                # d-tiles 14,15 in one fp8 DoubleRow matmul (256-dim
                # pair contraction at ~1.8x bf16 rate)
                nc.tensor.matmul(
                    po[:, 0:cw],
                    xdr[:, :, :],
                    wdr_t[oq][:, :, h * cw:h * cw + cw],
                    start=False,
                    stop=True,
                    perf_mode=mybir.MatmulPerfMode.DoubleRow,
                )
                osb = outp.tile([P, 512], BF16, tag="osb")
                # GPSIMD cannot read PSUM, and DVE evictions would queue
                # behind the prefix chain (priority inversion starving
                # PSUM) -> all evictions on ACT; DVE only at the tail
                if s == NS - 1:
                    nc.vector.tensor_copy(osb[:, 0:cw], po[:, 0:cw])
                else:
                    nc.scalar.copy(osb[:, 0:cw], po[:, 0:cw])
                nc.sync.dma_start(
                    out_d[s * P:(s + 1) * P,
                          oq * 512 + h * cw:oq * 512 + (h + 1) * cw],
                    osb[:, 0:cw])
            if ci in mask_at:
                mask_at[ci]()

    nc.compile()
    return nc


_PROGRAM = None


def _get_program():
    global _PROGRAM
    if _PROGRAM is None:
        _PROGRAM = _build_program()
    return _PROGRAM


def _rel_count_threshold(k: int) -> float:
    kf = np.float32(k)
    thr = np.float32(0.3)
    for c in range(k + 2):
        if np.float32(c) / kf >= thr:
            return float(c)
    return float(k + 1)


def _make_in_maps(x, boundaries, weight, bias, new_weight, permutation,
                  saved_masks, bg_mean, top_k):
    bf16 = ml_dtypes.bfloat16
    fp8 = ml_dtypes.float8_e4m3
    x = np.asarray(x, dtype=np.float32)
    boundaries = np.asarray(boundaries)
    w = np.asarray(weight, dtype=np.float32)
    # wt2[p, (oq*NDB + j)*512 + c] = W[oq*512 + c, j*P + p], j < NDB
    wt2 = np.ascontiguousarray(
        w.astype(bf16).reshape(NQ, 512, ND, P)[:, :, :NDB]
        .transpose(3, 0, 2, 1).reshape(P, NQ * SLAB))
    # wdr[p, (oq*2 + i)*512 + n] = fp8(W[oq*512 + n, (NDB+i)*P + p] / a)
    wdr = np.ascontiguousarray(
        (w / DR_SCALE).astype(fp8).reshape(NQ, 512, ND, P)[:, :, NDB:]
        .transpose(3, 0, 2, 1).reshape(P, NQ * 2 * 512))
    bg = np.ascontiguousarray(
        np.asarray(bg_mean, dtype=np.float32).reshape(1, D))
    perm = np.asarray(permutation).astype(np.int64)
    saved = np.asarray(saved_masks).astype(np.float32)        # [M, D]
    # savedT[p, t*128+m]  = saved[m, t*128+p]
    savedT = np.ascontiguousarray(
        saved.T.reshape(ND, P, M).transpose(1, 0, 2).reshape(P, ND * M)
        .astype(bf16))
    # savedPT[p, t*128+m] = saved[m, perm[t*128+p]]
    savedPT = np.ascontiguousarray(
        saved[:, perm].T.reshape(ND, P, M).transpose(1, 0, 2)
        .reshape(P, ND * M).astype(bf16))
    HI0 = 1024.0
    fracs = np.ascontiguousarray(np.array(
        [[(k + 1) / (NWAY + 1) * HI0 / (NWAY + 1) ** it
          for it in range(SEARCH_ITERS) for k in range(NWAY)]],
        dtype=np.float32))
    mrow = np.ascontiguousarray(
        (np.arange(M, dtype=np.float32) - 4096.0).reshape(1, M))
    k = int(top_k)
    relc = _rel_count_threshold(k)

    in_maps = []
    for i in range(N_CORES):
        bnd = float(np.clip(int(boundaries[i]), 0, S - 1))
        scal = np.array(
            [[bnd, bnd + 1.0, k - 0.5, 0.0, relc - 0.5, 0.0, 0.0, 0.0]],
            dtype=np.float32)
        # xT[p, s, j, c] = x[s*P + c, j*P + p], j < NDB
        xt = np.ascontiguousarray(
            x[i].astype(bf16).reshape(NS, P, ND, P)[:, :, :NDB]
            .transpose(3, 0, 2, 1).reshape(P, NS * NDB * P))
        # xdr[p, s, i, c] = fp8(a * x[s*P + c, (NDB+i)*P + p])
        xdr = np.ascontiguousarray(
            (x[i] * DR_SCALE).astype(fp8).reshape(NS, P, ND, P)[:, :, NDB:]
            .transpose(3, 0, 2, 1).reshape(P, NS * 2 * P))
        x8t = np.ascontiguousarray(
            x[i].astype(fp8).reshape(NS, P, ND, P).transpose(3, 0, 2, 1)
            .reshape(P, NS * ND * P))
        in_maps.append({
            "xT": xt,
            "xdr": xdr,
            "x8T": x8t,
            "wt2": wt2,
            "bg_r": bg,
            "savedT": savedT,
            "savedPT": savedPT,
            "wdr": wdr,
            "fracs": fracs,
            "mrow": mrow,
            "scal": scal,
        })
    return in_maps


def run(inputs: dict, trace: bool = False):
    nc = _get_program()
    in_maps = _make_in_maps(**inputs)
    res = run_bass_kernel_spmd(
        nc, in_maps, core_ids=list(range(N_CORES)), trace=trace)
    bias = np.asarray(inputs["bias"], dtype=np.float32)
    nw = np.asarray(inputs["new_weight"], dtype=np.float32)
    x = np.asarray(inputs["x"], dtype=np.float32)
    outs = []
    for i in range(N_CORES):
        o = np.asarray(res.results[i]["out"]).astype(np.float32) + bias
        aux = np.asarray(res.results[i]["aux"]).astype(np.float32)
        if aux[0, ND] != 0.0:
            # relevant: add the masked new-weight path (host f32, exact)
            fm = aux[:, 0:ND].T.reshape(D)  # fm[t*128+p] = aux[p, t]
            o = o + (x[i] * fm[None, :]) @ nw.T
        outs.append(o)
    return np.stack(outs, axis=0), res


def kernel(**inputs) -> np.ndarray:
    out, _ = run(inputs, trace=False)
    return out
